# revision 7
# baseline (speedup 1.0000x reference)
"""Llama-style GQA attention (S=4096, H=2048, 16 q heads / 4 kv heads, d=128, fp32)
on 8 Trainium2 NeuronCores.

Sharding: 4 head-groups x 2 sequence-halves. Core c = 2*g + sh owns q heads
[4g, 4g+4) (one kv head g) and query rows [2048*sh, 2048*(sh+1)). Each core
computes its partial o_proj output transposed ([out_feat, seq]) in bf16; the
host sums the 4 head-group partials per sequence half and concatenates.

v3 (vs v2, 478us): the attention phase is ACT(exp)-paced (~18.4us/tile vs
~13.8us of PE work), while the projection phase was PE-dense with ACT idle.
v3 fuses them so PE never idles and exp hides entirely under matmul work:
  - startup: chunk-0 K/V/Q0..3 projections interleaved per-ht-group across
    6 PSUM banks (psq from the psS pool's slots), fed by fine-grained DMA
    splits (weights' first-ht blocks first, wq in qd-major layout)
  - K/V projections of chunks 1-7 emitted as PE filler INSIDE attention
    tile (0,0), one chunk after each idx step (softmax key order is
    consumption order, so chunk c is produced right before idx c consumes
    it); Q projections of chunks 1-3 (one 16-MM unit per q head) spread as
    filler across tiles (0,1)..(2,1), reading hst re-loads (hspq, bufs=1)
    so the hsp streaming pool stays 3 deep
  - V transposed via dma_start_transpose (DMA xbar) instead of 32 PE
    transposes + 32 ACT evac copies
  - tail o_proj of the last tile rotates pso across 4 PSUM banks (psP x2 +
    a psS tile's halves) so unit starts never wait on the evac CAST
  - o_proj of sqt t-1 interleaved into attention of sqt t as before

Measured v2: 478us total, PE busy 419us (87.9%), ACT 296us, DVE 290us; PE
idle = 18us startup DMA ramp + ~0.8us/tile ACT-pacing stalls + 15us tail.
v3 targets ~410us (PE work ~390us + ~9us fixed preamble + drain).
Rejected: fp8 anywhere (rel_absmax 0.024-0.08 > 2e-2 tolerance), pair-wise
K/V AllGather dedup (2-rank 1MB collective costs ~100us), GPSIMD for acc
adds (shared SBUF port with DVE, ~2x slower), 16-bit PSUM (TRN3-only).
"""

import math

import numpy as np
import ml_dtypes

_S, _H, _HD = 4096, 2048, 128
_NCORES = 8
_SQ = _S // 2          # per-core query rows (2048)
_BF16 = ml_dtypes.bfloat16


def _build_nc():
    import concourse.bacc as bacc
    import concourse.mybir as mybir
    import concourse.tile as tile

    dt = mybir.dt
    F32, BF16, F16 = dt.float32, dt.bfloat16, dt.float16
    AF = mybir.ActivationFunctionType

    nc = bacc.Bacc("TRN2", target_bir_lowering=False, debug=False,
                   num_devices=_NCORES)

    def din(name, shape, dtype):
        return nc.dram_tensor(name, shape, dtype, kind="ExternalInput").ap()

    hs_l = din("hs_l", [128, 16 * 4096], BF16)     # hsT h-blocked, full seq
    wq_l = din("wq_l", [128, 4 * 2048], BF16)      # wqT qd-major (pre-scaled)
    wk_l = din("wk_l", [128, 16 * 128], BF16)
    wv_l = din("wv_l", [128, 16 * 128], BF16)
    wo_l = din("wo_l", [128, 4 * 2048], BF16)      # woT hd-blocked
    cos_k = din("cos_k", [128, 4096], F16)
    sinm_k = din("sinm_k", [128, 4096], F16)
    onesb = din("onesb", [128, 128], BF16)         # all-ones (denom broadcast)
    outT = nc.dram_tensor("outT", [2048, 2048], BF16, kind="ExternalOutput").ap()

    # One SPMD program for all cores: each core's hs/cos/sin columns are
    # host-permuted so its own sequence half comes first. Keys/values then
    # live in permuted order (softmax and attn@V sums are order-invariant),
    # and chunks 0..3 are always both the K/V-own and the Q chunks.

    with tile.TileContext(nc) as tc:
        with (
            tc.tile_pool(name="wp", bufs=1) as wp,
            tc.tile_pool(name="bigp", bufs=1) as bigp,
            tc.tile_pool(name="hsp", bufs=3) as hsp,
            tc.tile_pool(name="hqp", bufs=1) as hqp,
            tc.tile_pool(name="vtt", bufs=2) as vttp,
            tc.tile_pool(name="ptp", bufs=3) as ptp,
            tc.tile_pool(name="accp", bufs=2) as accp,
            tc.tile_pool(name="attnp", bufs=2) as attnp,
            tc.tile_pool(name="outp", bufs=3) as outp,
            tc.tile_pool(name="tmpp", bufs=2) as tmpp,
            tc.tile_pool(name="rbp", bufs=2) as rbp,
            tc.tile_pool(name="psP", bufs=2, space="PSUM") as psP,    # 2 banks
            tc.tile_pool(name="psS", bufs=2, space="PSUM") as psS,    # 4 banks
            tc.tile_pool(name="psAV", bufs=2, space="PSUM") as psAV,  # 2 banks
        ):
            # ---- resident weights/tables.
            wk_sb = wp.tile([128, 16 * 128], BF16, name="wk_sb")
            wv_sb = wp.tile([128, 16 * 128], BF16, name="wv_sb")
            wq_sb = wp.tile([128, 4 * 2048], BF16, name="wq_sb")
            cosk_sb = wp.tile([128, 4096], F16, name="cosk_sb")
            sinmk_sb = wp.tile([128, 4096], F16, name="sinmk_sb")
            onesb_sb = wp.tile([128, 128], BF16, name="onesb_sb")
            wo_sb = wp.tile([128, 4 * 2048], BF16, name="wo_sb")

            # ---- persistent activations
            qr = bigp.tile([128, 4 * 2048], BF16, name="qr")    # [d, qh*2048+sq]
            kr = bigp.tile([128, 4096], BF16, name="kr")        # [d, sk]
            vsb = bigp.tile([128, 4096], BF16, name="vsb")      # [sk%128, jt*128+d]

            hs3 = hs_l.rearrange("p (t s) -> p t s", t=16)

            # -------- startup DMA schedule (see docstring). First-ht blocks
            # of every weight go first so the per-ht-group chunk-0 interleave
            # can start as soon as hst block 0 lands.
            nc.sync.dma_start(wk_sb[:, 0:128], wk_l[:, 0:128])
            nc.sync.dma_start(wv_sb[:, 0:128], wv_l[:, 0:128])
            for qd in range(4):
                nc.sync.dma_start(wq_sb[:, qd * 2048: qd * 2048 + 128],
                                  wq_l[:, qd * 2048: qd * 2048 + 128])
            nc.sync.dma_start(wk_sb[:, 128:1024], wk_l[:, 128:1024])
            nc.sync.dma_start(wv_sb[:, 128:1024], wv_l[:, 128:1024])
            nc.sync.dma_start(wk_sb[:, 1024:], wk_l[:, 1024:])
            nc.sync.dma_start(wv_sb[:, 1024:], wv_l[:, 1024:])

            # hst chunk 0 in 6 ht-group pieces on the scalar queue, with the
            # wq rest pieces interleaved to match per-ht consumption order
            hst0 = hsp.tile([128, 16 * 512], BF16, name="hst", tag="hst")
            h30 = hst0.rearrange("p (t s) -> p t s", t=16)
            GRP = [(0, 1), (1, 2), (2, 4), (4, 8), (8, 12), (12, 16)]

            def wq_piece(qd, a, b):
                nc.scalar.dma_start(
                    wq_sb[:, qd * 2048 + a * 128: qd * 2048 + b * 128],
                    wq_l[:, qd * 2048 + a * 128: qd * 2048 + b * 128])

            nc.scalar.dma_start(h30[:, 0:1, :], hs3[:, 0:1, 0:512])
            nc.scalar.dma_start(h30[:, 1:2, :], hs3[:, 1:2, 0:512])
            wq_piece(0, 1, 8)
            wq_piece(1, 1, 8)
            nc.scalar.dma_start(h30[:, 2:4, :], hs3[:, 2:4, 0:512])
            wq_piece(2, 1, 8)
            wq_piece(3, 1, 8)
            nc.scalar.dma_start(h30[:, 4:8, :], hs3[:, 4:8, 0:512])
            wq_piece(0, 8, 16)
            wq_piece(1, 8, 16)
            nc.scalar.dma_start(h30[:, 8:12, :], hs3[:, 8:12, 0:512])
            wq_piece(2, 8, 16)
            wq_piece(3, 8, 16)
            nc.scalar.dma_start(h30[:, 12:16, :], hs3[:, 12:16, 0:512])

            nc.sync.dma_start(cosk_sb[:, 0:2048], cos_k[:, 0:2048])
            nc.sync.dma_start(onesb_sb[:, :], onesb[:, :])
            nc.scalar.dma_start(sinmk_sb[:, 0:2048], sinm_k[:, 0:2048])
            nc.scalar.dma_start(sinmk_sb[:, 2048:], sinm_k[:, 2048:])
            nc.sync.dma_start(cosk_sb[:, 2048:], cos_k[:, 2048:])
            nc.sync.dma_start(wo_sb[:, :], wo_l[:, :])

            # hst chunks 1-7 stream through hsp (bufs=3); alternate queues
            hst_tiles = {0: hst0}

            def load_hst(c, engine):
                hst = hsp.tile([128, 16 * 512], BF16, name="hst", tag="hst")
                h3 = hst.rearrange("p (t s) -> p t s", t=16)
                engine.dma_start(h3[:, :, :], hs3[:, :, c * 512:(c + 1) * 512])
                hst_tiles[c] = hst
                return hst

            def rope(dst, ps, c0):
                # dst = ps * cos + swap_halves(ps) * sinm  (partition dim = d)
                t1 = tmpp.tile([128, 512], F32, name="t1", tag="t1")
                t2 = tmpp.tile([128, 512], F32, name="t2", tag="t2")
                nc.vector.tensor_mul(t1[:, :], ps[:, :], cosk_sb[:, c0:c0 + 512])
                nc.vector.tensor_mul(t2[0:64, :], ps[64:128, :],
                                     sinmk_sb[0:64, c0:c0 + 512])
                nc.vector.tensor_mul(t2[64:128, :], ps[0:64, :],
                                     sinmk_sb[64:128, c0:c0 + 512])
                nc.vector.tensor_add(dst, t1[:, :], t2[:, :])

            def v_evac(c, psv):
                # psv [d, 512 seq] -> vt (SBUF) -> vsb [s%128, 4x128 d] via
                # DMA xbar transpose (frees PE + ACT vs transpose matmuls)
                vt = vttp.tile([128, 512], BF16, name="vt", tag="vt")
                nc.scalar.copy(vt[:, :], psv[:, :])
                dst = vsb[:, c * 512:(c + 1) * 512].rearrange(
                    "p (di m) -> p di m", di=4)
                nc.sync.dma_start_transpose(dst, vt[:, :])

            # -------- chunk-0 projections, interleaved per ht-group across
            # 6 PSUM banks so PE tracks the hst0 DMA ramp
            psk0 = psP.tile([128, 512], F32, name="psk0", tag="psP")
            psv0 = psP.tile([128, 512], F32, name="psv0", tag="psP")
            psq01 = psS.tile([128, 1024], F32, name="psq01", tag="psS")
            psq23 = psS.tile([128, 1024], F32, name="psq23", tag="psS")
            warmed = False
            for a, b in GRP:
                for ht in range(a, b):
                    hsl = hst0[:, ht * 512:(ht + 1) * 512]
                    nc.tensor.matmul(psk0[:, :], wk_sb[:, ht * 128:(ht + 1) * 128],
                                     hsl, start=(ht == 0), stop=(ht == 15))
                    nc.tensor.matmul(psv0[:, :], wv_sb[:, ht * 128:(ht + 1) * 128],
                                     hsl, start=(ht == 0), stop=(ht == 15))
                    for qd in range(4):
                        dstq = (psq01 if qd < 2 else psq23)[
                            :, (qd % 2) * 512:(qd % 2) * 512 + 512]
                        nc.tensor.matmul(
                            dstq, wq_sb[:, qd * 2048 + ht * 128:
                                        qd * 2048 + (ht + 1) * 128],
                            hsl, start=(ht == 0), stop=(ht == 15))
                if not warmed:
                    # pre-warm the exp table set while ACT is idle
                    wrm = vttp.tile([128, 16], BF16, name="wrm", tag="wrm")
                    nc.scalar.activation(wrm[:, :], hst0[:, 0:16], AF.Exp)
                    warmed = True
            rope(kr[:, 0:512], psk0, 0)
            v_evac(0, psv0)
            for qd in range(4):
                srcq = (psq01 if qd < 2 else psq23)[
                    :, (qd % 2) * 512:(qd % 2) * 512 + 512]
                rope(qr[:, qd * 2048: qd * 2048 + 512], srcq, 0)

            # prefetch the streaming chunks
            load_hst(1, nc.sync)
            load_hst(2, nc.scalar)
            load_hst(3, nc.sync)

            # -------- unit emitters used as PE filler inside attention
            def kv_unit(c):
                hst = hst_tiles[c]
                psk = psP.tile([128, 512], F32, name="psk", tag="psP")
                for ht in range(16):
                    nc.tensor.matmul(psk[:, :],
                                     wk_sb[:, ht * 128:(ht + 1) * 128],
                                     hst[:, ht * 512:(ht + 1) * 512],
                                     start=(ht == 0), stop=(ht == 15))
                rope(kr[:, c * 512:(c + 1) * 512], psk, c * 512)
                psv = psP.tile([128, 512], F32, name="psv", tag="psP")
                for ht in range(16):
                    nc.tensor.matmul(psv[:, :],
                                     wv_sb[:, ht * 128:(ht + 1) * 128],
                                     hst[:, ht * 512:(ht + 1) * 512],
                                     start=(ht == 0), stop=(ht == 15))
                v_evac(c, psv)
                # prefetch 2 chunks ahead AFTER the v transpose so the 2MB
                # transfer doesn't delay it in-queue (hsp bufs=3; the slot's
                # WAR is released by kv_unit(c-1)'s matmuls, already done)
                if c + 2 <= 7 and c + 2 not in hst_tiles:
                    load_hst(c + 2, nc.scalar if c % 2 == 0 else nc.sync)

            hq_tiles = {}

            def load_hq(c):
                hq = hqp.tile([128, 16 * 512], BF16, name="hq", tag="hq")
                h3 = hq.rearrange("p (t s) -> p t s", t=16)
                nc.scalar.dma_start(h3[:, :, :], hs3[:, :, c * 512:(c + 1) * 512])
                hq_tiles[c] = hq

            def q_unit(c, qd):
                hq = hq_tiles[c]
                psq = psP.tile([128, 512], F32, name="psq", tag="psP")
                for ht in range(16):
                    nc.tensor.matmul(
                        psq[:, :],
                        wq_sb[:, qd * 2048 + ht * 128: qd * 2048 + (ht + 1) * 128],
                        hq[:, ht * 512:(ht + 1) * 512],
                        start=(ht == 0), stop=(ht == 15))
                rope(qr[:, qd * 2048 + c * 512: qd * 2048 + (c + 1) * 512],
                     psq, c * 512)

            # -------- attention + o_proj + filler, single interleaved stream
            at_tiles = {}
            pending = None  # (h_slice_dst, psav, acc)

            def emit_norm(p):
                dst, psav, acc = p
                # fold the two acc halves on DVE so psd needs 2 MMs, not 4
                nc.vector.tensor_add(acc[:, 0:1024], acc[:, 0:1024],
                                     acc[:, 1024:2048])
                psd = psP.tile([128, 512], F32, name="psd", tag="psP")
                for q in range(2):
                    nc.tensor.matmul(psd[:, :], onesb_sb[:, :],
                                     acc[:, q * 512:(q + 1) * 512],
                                     start=(q == 0), stop=(q == 1))
                rb = rbp.tile([128, 512], F32, name="rb", tag="rb")
                nc.vector.reciprocal_approx_fast(rb[:, :], psd[:, :])
                nc.vector.tensor_mul(dst, psav[:, :], rb[:, :])

            def emit_oproj(t, ot, pso=None):
                at_t = at_tiles[t]
                if pso is None:
                    pso = psP.tile([128, 512], F32, name="pso", tag="psP")
                for hdt in range(4):
                    nc.tensor.matmul(
                        pso[:, :],
                        wo_sb[:, hdt * 2048 + ot * 128: hdt * 2048 + (ot + 1) * 128],
                        at_t[:, hdt * 512:(hdt + 1) * 512],
                        start=(hdt == 0), stop=(hdt == 3))
                osb = outp.tile([128, 512], BF16, name="osb", tag="osb")
                nc.vector.tensor_copy(osb[:, :], pso[:, :])
                nc.sync.dma_start(
                    outT[ot * 128:(ot + 1) * 128, t * 512:(t + 1) * 512],
                    osb[:, :])

            def attn_tile(t, h, fillers):
                """One (t,h) attention tile; fillers[j] (callable or None) is
                emitted after idx j's matmuls as ACT-pacing PE filler."""
                nonlocal pending
                qsl = qr[:, h * 2048 + t * 512: h * 2048 + (t + 1) * 512]
                psav = psAV.tile([128, 512], F32, name="psav", tag="psAV")
                acc = accp.tile([128, 2048], BF16, name="acc", tag="acc")
                for idx in range(8):  # 4 key-blocks (512 keys) per step
                    pt = ptp.tile([128, 2048], BF16, name="pt", tag="pt")
                    for half in range(2):
                        pss = psS.tile([128, 1024], F32, name="pss", tag="psS")
                        for j2 in range(2):
                            jt = 4 * idx + 2 * half + j2
                            nc.tensor.matmul(
                                pss[:, j2 * 512:(j2 + 1) * 512],
                                kr[:, jt * 128:(jt + 1) * 128], qsl,
                                start=True, stop=True)
                        nc.scalar.activation(
                            pt[:, half * 1024:(half + 1) * 1024],
                            pss[:, :], AF.Exp)
                        for j2 in range(2):
                            jt = 4 * idx + 2 * half + j2
                            nc.tensor.matmul(
                                psav[:, :],
                                vsb[:, jt * 128:(jt + 1) * 128],
                                pt[:, (2 * half + j2) * 512:
                                   (2 * half + j2 + 1) * 512],
                                start=(idx == 0 and half == 0 and j2 == 0),
                                stop=(idx == 7 and half == 1 and j2 == 1))
                    if idx == 0:
                        nc.vector.tensor_copy(acc[:, :], pt[:, :])
                    else:
                        nc.vector.tensor_add(acc[:, :], acc[:, :], pt[:, :])
                    if idx == 3 and pending is not None:
                        emit_norm(pending)
                        pending = None
                    f = fillers[idx] if idx < len(fillers) else None
                    if f is not None:
                        f()
                pending = (at_tiles[t][:, h * 512:(h + 1) * 512], psav, acc)

            # filler plan:
            #  (0,0): kv_unit(c+1) after idx c (chunk production = consumption)
            #  q_units spread so each hq reload (bufs=1) has a full tile
            #  between the previous chunk's last reader and its first use;
            #  t>=1 tiles also carry 4 o_proj units of sqt t-1
            q_sched = {
                (0, 1): [(1, 0), (1, 1)], (0, 2): [(1, 2), (1, 3)],
                (1, 0): [(2, 0), (2, 1)], (1, 1): [(2, 2), (2, 3)],
                (1, 3): [(3, 0), (3, 1)], (2, 0): [(3, 2), (3, 3)],
            }
            hq_load_at = {(0, 0): 1, (0, 3): 2, (1, 2): 3}

            for t in range(4):
                at_tiles[t] = attnp.tile([128, 4 * 512], BF16,
                                         name=f"at{t}", tag="at")
                for h in range(4):
                    fillers = [None] * 8
                    if t == 0 and h == 0:
                        for j in range(7):
                            fillers[j] = (lambda c=j + 1: kv_unit(c))
                    else:
                        slots = iter(range(8))
                        for (qc, qd) in q_sched.get((t, h), []):
                            s = next(slots)
                            fillers[s] = (lambda c=qc, q=qd: q_unit(c, q))
                        if t > 0:
                            # o_proj(t-1) needs norm(t-1,3), emitted at idx 3
                            # of (t,0) — for h==0 only use slots 4..7
                            for i, ot in enumerate(range(4 * h, 4 * h + 4)):
                                s = (4 + i) if h == 0 else next(slots)
                                fillers[s] = (lambda tt=t - 1, o=ot:
                                              emit_oproj(tt, o))
                    if hq_load_at.get((t, h)):
                        load_hq(hq_load_at[(t, h)])
                    attn_tile(t, h, fillers)

            # -------- tail: last norm + o_proj of sqt 3 with 4-way pso
            emit_norm(pending)
            pso_s = psS.tile([128, 1024], F32, name="pso_s", tag="psS")
            for ot in range(16):
                if ot % 4 < 2:
                    emit_oproj(3, ot)          # psP rotation (2 banks)
                else:
                    half = ot % 2
                    emit_oproj(3, ot, pso=pso_s[:, half * 512:(half + 1) * 512])

    nc.compile()
    return nc


def _blocks_p(x):
    """[(T*128), C] row-major -> [128, T*C] with block t at cols [t*C,(t+1)*C)."""
    t = x.shape[0] // 128
    return np.ascontiguousarray(
        x.reshape(t, 128, -1).transpose(1, 0, 2).reshape(128, -1))


def _prepare_in_maps(hidden_states, wq, wk, wv, wo):
    hs = np.ascontiguousarray(np.asarray(hidden_states, np.float32)[0])  # [S,H]
    hsT = np.ascontiguousarray(hs.T)                                     # [H,S]
    hsT_b = hsT.astype(_BF16)

    inv_freq = 1.0 / (10000.0 ** (np.arange(0, _HD, 2, dtype=np.float32) / _HD))
    t = np.arange(_S, dtype=np.float32)
    freqs = np.einsum("i,j->ij", t, inv_freq)
    emb = np.concatenate([freqs, freqs], axis=-1)                        # [S,128]
    cosT = np.ascontiguousarray(np.cos(emb).T.astype(np.float16))         # [128,S]
    sinm = np.sin(emb).astype(np.float32)
    sinm[:, :64] *= -1.0
    sinmT = np.ascontiguousarray(sinm.T.astype(np.float16))

    scale = 1.0 / math.sqrt(_HD)
    wq = np.asarray(wq, np.float32)
    wk = np.asarray(wk, np.float32)
    wv = np.asarray(wv, np.float32)
    wo = np.asarray(wo, np.float32)

    onesb = np.ones((128, 128), np.float32).astype(_BF16)

    in_maps = []
    for c in range(_NCORES):
        g, sh = c // 2, c % 2
        # key-order permutation: own seq half first (order-invariant for
        # softmax/attn sums; queries are never permuted)
        if sh == 0:
            hs_perm = hsT_b
            cos_p, sinm_p = cosT, sinmT
        else:
            hs_perm = np.concatenate(
                [hsT_b[:, _SQ:], hsT_b[:, :_SQ]], axis=1)
            cos_p = np.ascontiguousarray(
                np.concatenate([cosT[:, _SQ:], cosT[:, :_SQ]], axis=1))
            sinm_p = np.ascontiguousarray(
                np.concatenate([sinmT[:, _SQ:], sinmT[:, :_SQ]], axis=1))
        # wq qd-major: block qd = head qd of this group, h-blocked inside
        wq_g = (wq[512 * g:512 * (g + 1), :].T * scale).astype(_BF16)  # [H,512]
        wq_qd = np.concatenate(
            [_blocks_p(np.ascontiguousarray(wq_g[:, qd * 128:(qd + 1) * 128]))
             for qd in range(4)], axis=1)                              # [128,4*2048]
        in_maps.append({
            "hs_l": _blocks_p(np.ascontiguousarray(hs_perm)),
            "wq_l": wq_qd,
            "wk_l": _blocks_p(wk[128 * g:128 * (g + 1), :].T.astype(_BF16)),
            "wv_l": _blocks_p(wv[128 * g:128 * (g + 1), :].T.astype(_BF16)),
            "wo_l": _blocks_p(
                np.ascontiguousarray(wo[:, 512 * g:512 * (g + 1)].T).astype(_BF16)),
            "cos_k": cos_p,
            "sinm_k": sinm_p,
            "onesb": onesb,
        })
    return in_maps


def _run(inputs, trace=False):
    from concourse.bass_utils import run_bass_kernel_spmd

    nc = _build_nc()
    in_maps = _prepare_in_maps(**inputs)
    res = run_bass_kernel_spmd(nc, in_maps, core_ids=list(range(_NCORES)),
                               trace=trace)
    halves = []
    for sh in range(2):
        acc = np.zeros((2048, 2048), np.float32)
        for g in range(4):
            acc += np.asarray(res.results[2 * g + sh]["outT"], np.float32)
        halves.append(acc.T)
    out = np.concatenate(halves, axis=0)[None]
    return np.ascontiguousarray(out, dtype=np.float32), res


def kernel(**inputs):
    out, _ = _run(inputs, trace=False)
    return out


# revision 16
# speedup vs baseline: 1.0174x; 1.0174x over previous
"""Llama-style GQA attention (S=4096, H=2048, 16 q heads / 4 kv heads, d=128, fp32)
on 8 Trainium2 NeuronCores.

Sharding: 4 head-groups x 2 sequence-halves. Core c = 2*g + sh owns q heads
[4g, 4g+4) (one kv head g) and query rows [2048*sh, 2048*(sh+1)). Each core
computes its partial o_proj output transposed ([out_feat, seq]) in bf16; the
host sums the 4 head-group partials per sequence half and concatenates.

v3 (vs v2, 478us): the attention phase is ACT(exp)-paced (~18.4us/tile vs
~13.8us of PE work), while the projection phase was PE-dense with ACT idle.
v3 fuses them so PE never idles and exp hides entirely under matmul work:
  - startup: chunk-0 K/V/Q0..3 projections interleaved per-ht-group across
    6 PSUM banks (psq from the psS pool's slots), fed by fine-grained DMA
    splits (weights' first-ht blocks first, wq in qd-major layout)
  - K/V projections of chunks 1-7 emitted as PE filler INSIDE attention
    tile (0,0), one chunk after each idx step (softmax key order is
    consumption order, so chunk c is produced right before idx c consumes
    it); Q projections of chunks 1-3 (one 16-MM unit per q head) spread as
    filler across tiles (0,1)..(2,1), reading hst re-loads (hspq, bufs=1)
    so the hsp streaming pool stays 3 deep
  - V transposed via dma_start_transpose (DMA xbar) instead of 32 PE
    transposes + 32 ACT evac copies
  - tail o_proj of the last tile rotates pso across 4 PSUM banks (psP x2 +
    a psS tile's halves) so unit starts never wait on the evac CAST
  - o_proj of sqt t-1 interleaved into attention of sqt t as before

Measured v2: 478us total, PE busy 419us (87.9%), ACT 296us, DVE 290us; PE
idle = 18us startup DMA ramp + ~0.8us/tile ACT-pacing stalls + 15us tail.
v3 targets ~410us (PE work ~390us + ~9us fixed preamble + drain).
Rejected: fp8 anywhere (rel_absmax 0.024-0.08 > 2e-2 tolerance), pair-wise
K/V AllGather dedup (2-rank 1MB collective costs ~100us), GPSIMD for acc
adds (shared SBUF port with DVE, ~2x slower), 16-bit PSUM (TRN3-only).
"""

import math

import numpy as np
import ml_dtypes

_S, _H, _HD = 4096, 2048, 128
_NCORES = 8
_SQ = _S // 2          # per-core query rows (2048)
_BF16 = ml_dtypes.bfloat16


def _build_nc():
    import concourse.bacc as bacc
    import concourse.mybir as mybir
    import concourse.tile as tile

    dt = mybir.dt
    F32, BF16, F16 = dt.float32, dt.bfloat16, dt.float16
    AF = mybir.ActivationFunctionType

    nc = bacc.Bacc("TRN2", target_bir_lowering=False, debug=False,
                   num_devices=_NCORES)

    def din(name, shape, dtype):
        return nc.dram_tensor(name, shape, dtype, kind="ExternalInput").ap()

    hs_l = din("hs_l", [128, 16 * 4096], BF16)     # hsT h-blocked, full seq
    wq_l = din("wq_l", [128, 4 * 2048], BF16)      # wqT qd-major (pre-scaled)
    wk_l = din("wk_l", [128, 16 * 128], BF16)
    wv_l = din("wv_l", [128, 16 * 128], BF16)
    wo_l = din("wo_l", [128, 4 * 2048], BF16)      # woT hd-blocked
    cos_k = din("cos_k", [128, 4096], F16)
    sinm_k = din("sinm_k", [128, 4096], F16)
    onesb = din("onesb", [128, 128], BF16)         # all-ones (denom broadcast)
    # t-major output blocks: outT[t*2048 + feat, s] = partial_out[feat, t*512+s]
    # so each [128, 512] store is fully contiguous in DRAM (~3x store speed)
    outT = nc.dram_tensor("outT", [4 * 2048, 512], BF16, kind="ExternalOutput").ap()

    # One SPMD program for all cores: each core's hs/cos/sin columns are
    # host-permuted so its own sequence half comes first. Keys/values then
    # live in permuted order (softmax and attn@V sums are order-invariant),
    # and chunks 0..3 are always both the K/V-own and the Q chunks.

    with tile.TileContext(nc) as tc:
        with (
            tc.tile_pool(name="wp", bufs=1) as wp,
            tc.tile_pool(name="bigp", bufs=1) as bigp,
            tc.tile_pool(name="hsp", bufs=3) as hsp,
            tc.tile_pool(name="hqp", bufs=1) as hqp,
            tc.tile_pool(name="vtt", bufs=2) as vttp,
            tc.tile_pool(name="ptp", bufs=3) as ptp,
            tc.tile_pool(name="accp", bufs=2) as accp,
            tc.tile_pool(name="attnp", bufs=2) as attnp,
            tc.tile_pool(name="outp", bufs=6) as outp,
            tc.tile_pool(name="tmpp", bufs=2) as tmpp,
            tc.tile_pool(name="rbp", bufs=2) as rbp,
            tc.tile_pool(name="psP", bufs=2, space="PSUM") as psP,    # 2 banks
            tc.tile_pool(name="psS", bufs=2, space="PSUM") as psS,    # 4 banks
            tc.tile_pool(name="psAV", bufs=2, space="PSUM") as psAV,  # 2 banks
        ):
            # ---- resident weights/tables.
            wk_sb = wp.tile([128, 16 * 128], BF16, name="wk_sb")
            wv_sb = wp.tile([128, 16 * 128], BF16, name="wv_sb")
            wq_sb = wp.tile([128, 4 * 2048], BF16, name="wq_sb")
            cosk_sb = wp.tile([128, 4096], F16, name="cosk_sb")
            sinmk_sb = wp.tile([128, 4096], F16, name="sinmk_sb")
            onesb_sb = wp.tile([128, 128], BF16, name="onesb_sb")
            wo_sb = wp.tile([128, 4 * 2048], BF16, name="wo_sb")

            # ---- persistent activations
            qr = bigp.tile([128, 4 * 2048], BF16, name="qr")    # [d, qh*2048+sq]
            kr = bigp.tile([128, 4096], BF16, name="kr")        # [d, sk]
            vsb = bigp.tile([128, 4096], BF16, name="vsb")      # [sk%128, jt*128+d]

            hs3 = hs_l.rearrange("p (t s) -> p t s", t=16)

            # -------- startup DMA schedule (see docstring). First-ht blocks
            # of every weight go first so the per-ht-group chunk-0 interleave
            # can start as soon as hst block 0 lands.
            # hst chunk 0 in 6 ht-group pieces split across BOTH queues,
            # with weight pieces interleaved in per-ht consumption order
            hst0 = hsp.tile([128, 16 * 512], BF16, name="hst", tag="hst")
            h30 = hst0.rearrange("p (t s) -> p t s", t=16)
            GRP = [(0, 1), (1, 2), (2, 4), (4, 8), (8, 12), (12, 16)]

            def wq_piece(qd, a, b):
                nc.scalar.dma_start(
                    wq_sb[:, qd * 2048 + a * 128: qd * 2048 + b * 128],
                    wq_l[:, qd * 2048 + a * 128: qd * 2048 + b * 128])

            def hst0_grp(g, engine):
                a, b = GRP[g]
                engine.dma_start(h30[:, a:b, :], hs3[:, a:b, 0:512])

            # sync: first-ht weight blocks, then wk/wv rests + hst0 odd groups
            nc.sync.dma_start(wk_sb[:, 0:128], wk_l[:, 0:128])
            nc.sync.dma_start(wv_sb[:, 0:128], wv_l[:, 0:128])
            for qd in range(4):
                nc.sync.dma_start(wq_sb[:, qd * 2048: qd * 2048 + 128],
                                  wq_l[:, qd * 2048: qd * 2048 + 128])
            hst0_grp(1, nc.sync)
            nc.sync.dma_start(wk_sb[:, 128:1024], wk_l[:, 128:1024])
            nc.sync.dma_start(wv_sb[:, 128:1024], wv_l[:, 128:1024])
            hst0_grp(3, nc.sync)
            nc.sync.dma_start(wk_sb[:, 1024:], wk_l[:, 1024:])
            nc.sync.dma_start(wv_sb[:, 1024:], wv_l[:, 1024:])
            hst0_grp(5, nc.sync)
            nc.sync.dma_start(cosk_sb[:, 0:2048], cos_k[:, 0:2048])
            nc.sync.dma_start(onesb_sb[:, :], onesb[:, :])
            nc.sync.dma_start(cosk_sb[:, 2048:], cos_k[:, 2048:])
            nc.sync.dma_start(wo_sb[:, :], wo_l[:, :])

            # scalar: hst0 even groups interleaved with wq rests
            hst0_grp(0, nc.scalar)
            wq_piece(0, 1, 8)
            wq_piece(1, 1, 8)
            hst0_grp(2, nc.scalar)
            wq_piece(2, 1, 8)
            wq_piece(3, 1, 8)
            hst0_grp(4, nc.scalar)
            wq_piece(0, 8, 16)
            wq_piece(1, 8, 16)
            wq_piece(2, 8, 16)
            wq_piece(3, 8, 16)
            nc.scalar.dma_start(sinmk_sb[:, 0:2048], sinm_k[:, 0:2048])
            nc.scalar.dma_start(sinmk_sb[:, 2048:], sinm_k[:, 2048:])

            # hst chunks 1-7 stream through hsp (bufs=3); alternate queues
            hst_tiles = {0: hst0}

            def load_hst(c, engine):
                hst = hsp.tile([128, 16 * 512], BF16, name="hst", tag="hst")
                h3 = hst.rearrange("p (t s) -> p t s", t=16)
                engine.dma_start(h3[:, :, :], hs3[:, :, c * 512:(c + 1) * 512])
                hst_tiles[c] = hst
                return hst

            def rope(dst, ps, c0):
                # dst = ps * cos + swap_halves(ps) * sinm  (partition dim = d)
                t1 = tmpp.tile([128, 512], F32, name="t1", tag="t1")
                t2 = tmpp.tile([128, 512], F32, name="t2", tag="t2")
                nc.vector.tensor_mul(t1[:, :], ps[:, :], cosk_sb[:, c0:c0 + 512])
                nc.vector.tensor_mul(t2[0:64, :], ps[64:128, :],
                                     sinmk_sb[0:64, c0:c0 + 512])
                nc.vector.tensor_mul(t2[64:128, :], ps[0:64, :],
                                     sinmk_sb[64:128, c0:c0 + 512])
                nc.vector.tensor_add(dst, t1[:, :], t2[:, :])

            def v_evac(c, psv):
                # psv [d, 512 seq] -> vt (SBUF) -> vsb [s%128, 4x128 d] via
                # DMA xbar transpose (frees PE + ACT vs transpose matmuls)
                vt = vttp.tile([128, 512], BF16, name="vt", tag="vt")
                nc.scalar.copy(vt[:, :], psv[:, :])
                dst = vsb[:, c * 512:(c + 1) * 512].rearrange(
                    "p (di m) -> p di m", di=4)
                nc.sync.dma_start_transpose(dst, vt[:, :])

            # -------- chunk-0 projections, interleaved per ht-group across
            # 6 PSUM banks so PE tracks the hst0 DMA ramp
            psk0 = psP.tile([128, 512], F32, name="psk0", tag="psP")
            psv0 = psP.tile([128, 512], F32, name="psv0", tag="psP")
            psq01 = psS.tile([128, 1024], F32, name="psq01", tag="psS")
            psq23 = psS.tile([128, 1024], F32, name="psq23", tag="psS")
            warmed = False
            for a, b in GRP:
                for ht in range(a, b):
                    hsl = hst0[:, ht * 512:(ht + 1) * 512]
                    nc.tensor.matmul(psk0[:, :], wk_sb[:, ht * 128:(ht + 1) * 128],
                                     hsl, start=(ht == 0), stop=(ht == 15))
                    nc.tensor.matmul(psv0[:, :], wv_sb[:, ht * 128:(ht + 1) * 128],
                                     hsl, start=(ht == 0), stop=(ht == 15))
                    for qd in range(4):
                        dstq = (psq01 if qd < 2 else psq23)[
                            :, (qd % 2) * 512:(qd % 2) * 512 + 512]
                        nc.tensor.matmul(
                            dstq, wq_sb[:, qd * 2048 + ht * 128:
                                        qd * 2048 + (ht + 1) * 128],
                            hsl, start=(ht == 0), stop=(ht == 15))
                if not warmed:
                    # pre-warm the exp table set while ACT is idle
                    wrm = vttp.tile([128, 16], BF16, name="wrm", tag="wrm")
                    nc.scalar.activation(wrm[:, :], hst0[:, 0:16], AF.Exp)
                    warmed = True
            rope(kr[:, 0:512], psk0, 0)
            v_evac(0, psv0)
            for qd in range(4):
                srcq = (psq01 if qd < 2 else psq23)[
                    :, (qd % 2) * 512:(qd % 2) * 512 + 512]
                rope(qr[:, qd * 2048: qd * 2048 + 512], srcq, 0)

            # prefetch the streaming chunks
            load_hst(1, nc.sync)
            load_hst(2, nc.scalar)
            load_hst(3, nc.sync)

            # -------- unit emitters used as PE filler inside attention
            def kv_k_part(c):
                hst = hst_tiles[c]
                psk = psP.tile([128, 512], F32, name="psk", tag="psP")
                for ht in range(16):
                    nc.tensor.matmul(psk[:, :],
                                     wk_sb[:, ht * 128:(ht + 1) * 128],
                                     hst[:, ht * 512:(ht + 1) * 512],
                                     start=(ht == 0), stop=(ht == 15))
                rope(kr[:, c * 512:(c + 1) * 512], psk, c * 512)

            def kv_v_part(c):
                hst = hst_tiles[c]
                psv = psP.tile([128, 512], F32, name="psv", tag="psP")
                for ht in range(16):
                    nc.tensor.matmul(psv[:, :],
                                     wv_sb[:, ht * 128:(ht + 1) * 128],
                                     hst[:, ht * 512:(ht + 1) * 512],
                                     start=(ht == 0), stop=(ht == 15))
                v_evac(c, psv)
                # prefetch 2 chunks ahead AFTER the v transpose so the 2MB
                # transfer doesn't delay it in-queue (hsp bufs=3; the slot's
                # WAR is released by kv matmuls 3 chunks back, already done)
                if c + 2 <= 7 and c + 2 not in hst_tiles:
                    load_hst(c + 2, nc.scalar if c % 2 == 0 else nc.sync)

            hq_tiles = {}

            def load_hq(c):
                hq = hqp.tile([128, 16 * 512], BF16, name="hq", tag="hq")
                h3 = hq.rearrange("p (t s) -> p t s", t=16)
                nc.scalar.dma_start(h3[:, :, :], hs3[:, :, c * 512:(c + 1) * 512])
                hq_tiles[c] = hq

            def q_unit(c, qd):
                hq = hq_tiles[c]
                psq = psP.tile([128, 512], F32, name="psq", tag="psP")
                for ht in range(16):
                    nc.tensor.matmul(
                        psq[:, :],
                        wq_sb[:, qd * 2048 + ht * 128: qd * 2048 + (ht + 1) * 128],
                        hq[:, ht * 512:(ht + 1) * 512],
                        start=(ht == 0), stop=(ht == 15))
                rope(qr[:, qd * 2048 + c * 512: qd * 2048 + (c + 1) * 512],
                     psq, c * 512)

            # -------- attention + o_proj + filler, single interleaved stream
            at_tiles = {}
            pending = None  # (h_slice_dst, psav, acc)

            def emit_norm(p):
                dst, psav, acc = p
                # fold the two acc halves on DVE so psd needs 2 MMs, not 4
                nc.vector.tensor_add(acc[:, 0:1024], acc[:, 0:1024],
                                     acc[:, 1024:2048])
                psd = psP.tile([128, 512], F32, name="psd", tag="psP")
                for q in range(2):
                    nc.tensor.matmul(psd[:, :], onesb_sb[:, :],
                                     acc[:, q * 512:(q + 1) * 512],
                                     start=(q == 0), stop=(q == 1))
                rb = rbp.tile([128, 512], F32, name="rb", tag="rb")
                nc.vector.reciprocal_approx_fast(rb[:, :], psd[:, :])
                nc.vector.tensor_mul(dst, psav[:, :], rb[:, :])

            def emit_oproj(t, ot, pso=None):
                at_t = at_tiles[t]
                if pso is None:
                    pso = psP.tile([128, 512], F32, name="pso", tag="psP")
                for hdt in range(4):
                    nc.tensor.matmul(
                        pso[:, :],
                        wo_sb[:, hdt * 2048 + ot * 128: hdt * 2048 + (ot + 1) * 128],
                        at_t[:, hdt * 512:(hdt + 1) * 512],
                        start=(hdt == 0), stop=(hdt == 3))
                osb = outp.tile([128, 512], BF16, name="osb", tag="osb")
                nc.vector.tensor_copy(osb[:, :], pso[:, :])
                nc.sync.dma_start(
                    outT[t * 2048 + ot * 128: t * 2048 + (ot + 1) * 128, :],
                    osb[:, :])

            def attn_tile(t, h, fillers, kv_split=None):
                """One (t,h) attention tile; fillers[j] (callable or None) is
                emitted after idx j's matmuls as ACT-pacing PE filler.
                kv_split[j] = (k_part, v_part): K proj of the next chunk goes
                between the exps and the attnV matmuls, V proj right after, so
                the rope / V-transpose land well before idx j+1 consumes them."""
                nonlocal pending
                qsl = qr[:, h * 2048 + t * 512: h * 2048 + (t + 1) * 512]
                psav = psAV.tile([128, 512], F32, name="psav", tag="psAV")
                acc = accp.tile([128, 2048], BF16, name="acc", tag="acc")
                for idx in range(8):  # 4 key-blocks (512 keys) per step
                    pt = ptp.tile([128, 2048], BF16, name="pt", tag="pt")
                    for half in range(2):
                        pss = psS.tile([128, 1024], F32, name="pss", tag="psS")
                        for j2 in range(2):
                            jt = 4 * idx + 2 * half + j2
                            nc.tensor.matmul(
                                pss[:, j2 * 512:(j2 + 1) * 512],
                                kr[:, jt * 128:(jt + 1) * 128], qsl,
                                start=True, stop=True)
                        nc.scalar.activation(
                            pt[:, half * 1024:(half + 1) * 1024],
                            pss[:, :], AF.Exp)
                    if kv_split is not None and kv_split.get(idx):
                        kv_split[idx][0]()
                    for half in range(2):
                        for j2 in range(2):
                            jt = 4 * idx + 2 * half + j2
                            nc.tensor.matmul(
                                psav[:, :],
                                vsb[:, jt * 128:(jt + 1) * 128],
                                pt[:, (2 * half + j2) * 512:
                                   (2 * half + j2 + 1) * 512],
                                start=(idx == 0 and half == 0 and j2 == 0),
                                stop=(idx == 7 and half == 1 and j2 == 1))
                    if kv_split is not None and kv_split.get(idx):
                        kv_split[idx][1]()
                    if idx == 0:
                        nc.vector.tensor_copy(acc[:, :], pt[:, :])
                    else:
                        nc.vector.tensor_add(acc[:, :], acc[:, :], pt[:, :])
                    if idx == 3 and pending is not None:
                        emit_norm(pending)
                        pending = None
                    f = fillers[idx] if idx < len(fillers) else None
                    if f is not None:
                        f()
                pending = (at_tiles[t][:, h * 512:(h + 1) * 512], psav, acc)

            # filler plan:
            #  (0,0): kv_unit(c+1) after idx c (chunk production = consumption)
            #  q_units spread so each hq reload (bufs=1) has a full tile
            #  between the previous chunk's last reader and its first use;
            #  t>=1 tiles also carry 4 o_proj units of sqt t-1
            q_sched = {
                (0, 1): [(1, 0), (1, 1)], (0, 2): [(1, 2), (1, 3)],
                (1, 0): [(2, 0), (2, 1)], (1, 1): [(2, 2), (2, 3)],
                (1, 3): [(3, 0), (3, 1)], (2, 0): [(3, 2), (3, 3)],
            }
            hq_load_at = {(0, 0): 1, (0, 3): 2, (1, 2): 3}

            for t in range(4):
                at_tiles[t] = attnp.tile([128, 4 * 512], BF16,
                                         name=f"at{t}", tag="at")
                for h in range(4):
                    fillers = [None] * 8
                    kv_split = None
                    if t == 0 and h == 0:
                        kv_split = {j: ((lambda c=j + 1: kv_k_part(c)),
                                        (lambda c=j + 1: kv_v_part(c)))
                                    for j in range(7)}
                    else:
                        slots = iter(range(8))
                        for (qc, qd) in q_sched.get((t, h), []):
                            s = next(slots)
                            fillers[s] = (lambda c=qc, q=qd: q_unit(c, q))
                        if t > 0:
                            # o_proj(t-1) needs norm(t-1,3), emitted at idx 3
                            # of (t,0) — for h==0 only use slots 4..7
                            for i, ot in enumerate(range(4 * h, 4 * h + 4)):
                                s = (4 + i) if h == 0 else next(slots)
                                fillers[s] = (lambda tt=t - 1, o=ot:
                                              emit_oproj(tt, o))
                    if hq_load_at.get((t, h)):
                        load_hq(hq_load_at[(t, h)])
                    attn_tile(t, h, fillers, kv_split)

            # -------- tail: last norm + o_proj of sqt 3 with 4-way pso
            emit_norm(pending)
            pso_s = psS.tile([128, 1024], F32, name="pso_s", tag="psS")
            for ot in range(16):
                if ot % 4 < 2:
                    emit_oproj(3, ot)          # psP rotation (2 banks)
                else:
                    half = ot % 2
                    emit_oproj(3, ot, pso=pso_s[:, half * 512:(half + 1) * 512])

    nc.compile()
    return nc


def _blocks_p(x):
    """[(T*128), C] row-major -> [128, T*C] with block t at cols [t*C,(t+1)*C)."""
    t = x.shape[0] // 128
    return np.ascontiguousarray(
        x.reshape(t, 128, -1).transpose(1, 0, 2).reshape(128, -1))


def _prepare_in_maps(hidden_states, wq, wk, wv, wo):
    hs = np.ascontiguousarray(np.asarray(hidden_states, np.float32)[0])  # [S,H]
    hsT = np.ascontiguousarray(hs.T)                                     # [H,S]
    hsT_b = hsT.astype(_BF16)

    inv_freq = 1.0 / (10000.0 ** (np.arange(0, _HD, 2, dtype=np.float32) / _HD))
    t = np.arange(_S, dtype=np.float32)
    freqs = np.einsum("i,j->ij", t, inv_freq)
    emb = np.concatenate([freqs, freqs], axis=-1)                        # [S,128]
    cosT = np.ascontiguousarray(np.cos(emb).T.astype(np.float16))         # [128,S]
    sinm = np.sin(emb).astype(np.float32)
    sinm[:, :64] *= -1.0
    sinmT = np.ascontiguousarray(sinm.T.astype(np.float16))

    scale = 1.0 / math.sqrt(_HD)
    wq = np.asarray(wq, np.float32)
    wk = np.asarray(wk, np.float32)
    wv = np.asarray(wv, np.float32)
    wo = np.asarray(wo, np.float32)

    onesb = np.ones((128, 128), np.float32).astype(_BF16)

    in_maps = []
    for c in range(_NCORES):
        g, sh = c // 2, c % 2
        # key-order permutation: own seq half first (order-invariant for
        # softmax/attn sums; queries are never permuted)
        if sh == 0:
            hs_perm = hsT_b
            cos_p, sinm_p = cosT, sinmT
        else:
            hs_perm = np.concatenate(
                [hsT_b[:, _SQ:], hsT_b[:, :_SQ]], axis=1)
            cos_p = np.ascontiguousarray(
                np.concatenate([cosT[:, _SQ:], cosT[:, :_SQ]], axis=1))
            sinm_p = np.ascontiguousarray(
                np.concatenate([sinmT[:, _SQ:], sinmT[:, :_SQ]], axis=1))
        # wq qd-major: block qd = head qd of this group, h-blocked inside
        wq_g = (wq[512 * g:512 * (g + 1), :].T * scale).astype(_BF16)  # [H,512]
        wq_qd = np.concatenate(
            [_blocks_p(np.ascontiguousarray(wq_g[:, qd * 128:(qd + 1) * 128]))
             for qd in range(4)], axis=1)                              # [128,4*2048]
        in_maps.append({
            "hs_l": _blocks_p(np.ascontiguousarray(hs_perm)),
            "wq_l": wq_qd,
            "wk_l": _blocks_p(wk[128 * g:128 * (g + 1), :].T.astype(_BF16)),
            "wv_l": _blocks_p(wv[128 * g:128 * (g + 1), :].T.astype(_BF16)),
            "wo_l": _blocks_p(
                np.ascontiguousarray(wo[:, 512 * g:512 * (g + 1)].T).astype(_BF16)),
            "cos_k": cos_p,
            "sinm_k": sinm_p,
            "onesb": onesb,
        })
    return in_maps


def _run(inputs, trace=False):
    from concourse.bass_utils import run_bass_kernel_spmd

    nc = _build_nc()
    in_maps = _prepare_in_maps(**inputs)
    res = run_bass_kernel_spmd(nc, in_maps, core_ids=list(range(_NCORES)),
                               trace=trace)
    halves = []
    for sh in range(2):
        acc = np.zeros((4, 2048, 512), np.float32)
        for g in range(4):
            acc += np.asarray(res.results[2 * g + sh]["outT"],
                              np.float32).reshape(4, 2048, 512)
        # acc[t, feat, s] -> [feat, t*512+s] -> transpose to [seq, feat]
        halves.append(acc.transpose(0, 2, 1).reshape(2048, 2048))
    out = np.concatenate(halves, axis=0)[None]
    return np.ascontiguousarray(out, dtype=np.float32), res


def kernel(**inputs):
    out, _ = _run(inputs, trace=False)
    return out
